# revision 37
# baseline (speedup 1.0000x reference)
"""Trainium2 Bass kernel for the DGN message-passing network.

Computation (per batch item b):
    h = relu(x @ enc_w + enc_b)                      [N, H]
    for p in 0..P-1:
        v = relu(h @ wv[p] + bv[p]); q = relu(h @ wq[p] + bq[p]); k = relu(h @ wk[p] + bk[p])
        att = softmax(q @ k.T  masked by mask, axis=-1)
        h = relu((att @ v) @ wo[p] + bo[p])
    y = h @ qw + qb                                  [N, A]

Sharding: data-parallel over the batch dim across 8 NeuronCores (16 items
per core), weights replicated, no cross-core communication.

On-chip layout: everything is kept transposed ([H, N] with H on partitions)
so no transposes are ever required:
  * hT/qT/kT = [H=128, N=512];   projections:  qT = wq.T @ hT  (lhsT = wq)
  * scoresT[m, n] = q[n]*k[m] computed directly as kT-chunk.T @ qT
  * v is built natively m-on-partitions as 4 row chunks packed in one
    [128, 4*H] PSUM tile; the bias (free-axis-varying there) is preloaded
    with a single K=1 ones x bv4 matmul, the 4 h-chunk matmuls accumulate.

The mask is applied per m-chunk by whichever engine has slack, staggered
one of each per exp half so nothing serializes:
  * chunks 0,2: additive -30000*(1-mask) folded into the scores PSUM by an
    identity matmul (PE), so exp() yields exactly-masked weights.
  * chunks 1,3: multiplicative raw 0/1 mask applied to exp() output by a
    GPSIMD tensor_tensor each (the only op kind GPSIMD supports; it cannot
    touch PSUM, so it gets the pure-SBUF bf16 work). The rowsum/att@v
    matmuls consume chunks ident-first so they never wait on GPSIMD.

Softmax: no max-subtraction (q,k >= 0 post-relu so scores are O(10); exp
is safe in f32/bf16). 1/rowsum = exp(-ln(rowsum)) on ACT - Ln/Exp share a
table set, DVE's InstReciprocal measures 3.4us/tile (6.5 cyc/elem), and
custom-DVE approx ops are rejected by this walrus build.

Engine busy per core (measured at the fast device clock, span ~197us):
  PE : q,k MMs + v preload/4MM + 4 score MMs + 2 ident-mask MMs + 4
       rowsum MMs + 4 att@v MMs + wo MM (+ enc/head)        ~166us
  ACT: exp x2 [128,1024] + ln + exp(-ln) (+ enc-relu)       ~123us
  DVE: q,k relus + v-relu + otn-mult + h2-relu (+ y-add)    ~119us
  GPS: two [128,512] mask mults                             ~90us

Startup pitfalls encoded below: weight DMAs go on the GPSIMD trigger
queue (sync carries the big per-item mask/x transfers); ones/bv4 are
queued first because the scheduler hoists the dependency-light v-bias
preload matmul to the PE program head; a dummy activation warms the ACT
function table off the critical path; per-item mask DMAs are split
ident-chunks-first.

Pipeline: single-item stages [E, A0,B0,C0,D0, A1,B1,C1,D1, Hd], emitted
with a skew of 2 stage-slots between consecutive items (~5 in flight).
PSUM is exactly 8 banks: QK[128,512]x1 (enc,q,k,y rotate), SC[128,1024]x2
(score chunk-pairs double-buffered), V x1, RS x1, OT/H2 x1 (ot, h2
rotate - h2's matmul depends on otn reading ot, so the rotation is free).
"""

import numpy as np

import concourse.bass as bass
import concourse.mybir as mybir
import concourse.tile as tile
from concourse.bass import ts
from concourse.bass_utils import run_bass_kernel_spmd

F32 = mybir.dt.float32
BF16 = mybir.dt.bfloat16
AF = mybir.ActivationFunctionType
OP = mybir.AluOpType

N_CORES = 8
B, N, DIN, H, P, A = 128, 512, 64, 128, 2, 16
IPC = B // N_CORES  # batch items per core
NCH = N // 128      # 128-row chunks of the agent dim
# m-chunks masked additively via PE ident-matmul; the others get the raw
# 0/1 mask multiplied into exp() output on GPSIMD. One of each per exp
# half, so each GPSIMD op overlaps the adjacent score/rowsum matmuls.
IDENT_CHUNKS = (0, 2)
GPS_CHUNKS = tuple(c for c in range(NCH) if c not in IDENT_CHUNKS)


def _ident_set(i):
    # At the pipeline's ends the PE idles while the softmax chain hops
    # through GPSIMD; mask those items fully on the PE instead (shorter
    # chain, and the extra ident matmuls land in otherwise-idle PE time).
    return IDENT_CHUNKS


def _spill_excess_waits(nc):
    """Walrus codegen has limited sync-wait slots per instruction: a
    self-loading fp32/fp32r Matmult takes only 1 (waits land on its fused
    LDWEIGHTS micro-op) and sequencer ctrl ops (Drain/NoOp) take 4. Spill
    excess waits onto NoOps inserted just before the instruction on the same
    engine - the engine blocks at the NoOp, so ordering semantics are kept.
    """
    counter = [0]

    def make_nop(engine, waits):
        counter[0] += 1
        nop = mybir.InstNoOp(name=f"I-waitspill-{counter[0]}")
        nop.engine = engine
        nop.sync_info = mybir.SyncInfo(on_wait=list(waits), on_update=[])
        return nop

    def sem_clear_insts(inst):
        """This walrus build rejects EVENT_SEMAPHORE_RANGE_CLEAR ("ISA wrong
        length"); expand Tile's tail range-clear into per-sem writes."""
        first = inst.ant_dict["range_first"]
        last = inst.ant_dict["range_last"]
        res = []
        for s in range(first, last + 1):
            counter[0] += 1
            ev = mybir.InstEventSemaphore(name=f"I-semclear-{counter[0]}")
            ev.engine = inst.engine
            ev.sync_info = mybir.SyncInfo(
                on_wait=list(inst.sync_info.on_wait) if (s == first and inst.sync_info) else [],
                on_update=[mybir.SyncUpdate(
                    sync_type="semaphore", id=s,
                    update_mode="sem-wr-imm", update_value=0,
                )],
            )
            res.append(ev)
        return res

    for fn in nc.m.functions:
        for blk in fn.blocks:
            out = []
            for inst in blk.instructions:
                if (type(inst).__name__ == "InstISA"
                        and inst.ant_dict.get("header", {}).get("opcode") == 176):
                    out.extend(sem_clear_insts(inst))
                    continue
                si = inst.sync_info
                waits = list(si.on_wait) if si is not None else []
                limit = 1
                if len(waits) > limit:
                    keep = waits[-limit:] if limit else []
                    spill = waits[: len(waits) - limit]
                    for w in spill:
                        out.append(make_nop(inst.engine, [w]))
                    inst.sync_info.on_wait = keep
                out.append(inst)
            blk.instructions = out


def build_program():
    nc = bass.Bass("TRN2", target_bir_lowering=False, debug=False)

    xt_d = nc.dram_tensor("xt", [DIN, IPC * N], BF16, kind="ExternalInput").ap()
    # combined mask, pre-chunked to [128, NCH*N]: IDENT_CHUNKS cols hold
    # the additive -30000*(1-mask) form, GPS_CHUNKS cols raw 0/1
    mc_d = nc.dram_tensor("maskc", [IPC, 128, NCH * N], BF16, kind="ExternalInput").ap()
    encw_d = nc.dram_tensor("enc_w", [DIN, H], BF16, kind="ExternalInput").ap()
    encb_d = nc.dram_tensor("enc_b", [H, 1], F32, kind="ExternalInput").ap()
    wq_d = nc.dram_tensor("wq", [P, H, H], BF16, kind="ExternalInput").ap()
    wk_d = nc.dram_tensor("wk", [P, H, H], BF16, kind="ExternalInput").ap()
    wv_d = nc.dram_tensor("wv", [P, H, H], BF16, kind="ExternalInput").ap()
    wo_d = nc.dram_tensor("wo", [P, H, H], BF16, kind="ExternalInput").ap()
    bq_d = nc.dram_tensor("bq", [P, H, 1], F32, kind="ExternalInput").ap()
    bk_d = nc.dram_tensor("bk", [P, H, 1], F32, kind="ExternalInput").ap()
    bv4_d = nc.dram_tensor("bv4", [P, 1, NCH * H], BF16, kind="ExternalInput").ap()
    bo_d = nc.dram_tensor("bo", [P, H, 1], F32, kind="ExternalInput").ap()
    qw_d = nc.dram_tensor("qw", [H, A], BF16, kind="ExternalInput").ap()
    ones_d = nc.dram_tensor("ones", [128, 128], BF16, kind="ExternalInput").ap()
    ident_d = nc.dram_tensor("ident", [128, 128], BF16, kind="ExternalInput").ap()
    qb_d = nc.dram_tensor("qb", [A, 1], F32, kind="ExternalInput").ap()
    yt_d = nc.dram_tensor("yt", [IPC, A, N], F32, kind="ExternalOutput").ap()

    from contextlib import ExitStack

    with tile.TileContext(nc) as tc:
        with ExitStack() as stack:
            ep_ = lambda p: stack.enter_context(p)
            wpool = ep_(tc.tile_pool(name="weights", bufs=1))
            xpool = ep_(tc.tile_pool(name="xin", bufs=1))
            mcpool = ep_(tc.tile_pool(name="maskin", bufs=8))
            hpool = ep_(tc.tile_pool(name="hbuf", bufs=8))
            qpool = ep_(tc.tile_pool(name="qbuf", bufs=3))
            kpool = ep_(tc.tile_pool(name="kbuf", bufs=3))
            vpool = ep_(tc.tile_pool(name="vbuf", bufs=3))
            ppool = ep_(tc.tile_pool(name="pbuf", bufs=4))
            rpool = ep_(tc.tile_pool(name="rbuf", bufs=2))
            opool = ep_(tc.tile_pool(name="obuf", bufs=2))
            ypool = ep_(tc.tile_pool(name="ybuf", bufs=2))
            # PSUM: exactly 8 banks.
            qkpsum = ep_(tc.tile_pool(name="qkpsum", bufs=1, space="PSUM"))
            scpsum = ep_(tc.tile_pool(name="scpsum", bufs=2, space="PSUM"))
            vpsum = ep_(tc.tile_pool(name="vpsum", bufs=1, space="PSUM"))
            rspsum = ep_(tc.tile_pool(name="rspsum", bufs=1, space="PSUM"))
            otpsum = ep_(tc.tile_pool(name="otpsum", bufs=1, space="PSUM"))

            # ---- all items' x in one DMA (tiny; avoids 16 per-item
            # overhead-bound transfers) ----
            xall_t = xpool.tile([DIN, IPC * N], BF16, tag="xall")
            nc.sync.dma_start(out=xall_t[:], in_=xt_d[:])

            # ---- resident weights on the GPSIMD DMA queue (idle early),
            # streamed in FIRST-USE order. Tile waits are semaphore
            # thresholds, so a consumer waits for every earlier DMA on the
            # same queue - only enc_w/enc_b go before item 0's entry
            # stage; everything else trickles in behind it. ----
            def wtile(shape, dt, tag, dram, eng=None):
                t = wpool.tile(shape, dt, tag=tag)
                (eng or nc.gpsimd).dma_start(out=t[:], in_=dram)
                return t

            # warm the ACT function table before any real dependency: the
            # first ACTIVATE otherwise bundles a 1.3us table load onto the
            # critical enc-relu of item 0.
            warm_t = wpool.tile([1, 4], F32, tag="warm")
            nc.gpsimd.memset(warm_t[:], 0.0)
            nc.scalar.activation(warm_t[:], warm_t[:], AF.Exp)

            encw_t = wtile([DIN, H], BF16, "encw", encw_d[:])
            encb_t = wtile([H, 1], F32, "encb", encb_d[:])

            # per-item state
            st = [dict() for _ in range(IPC)]

            def stage_E(i):
                s = st[i]
                xt = xall_t[:, ts(i, N)]
                mc = mcpool.tile([128, NCH * N], BF16, tag="mc")
                # ident chunks land first: the score-PSUM ident matmuls are
                # the earliest mask consumers
                for cs in (IDENT_CHUNKS, GPS_CHUNKS):
                    for c in cs:
                        nc.sync.dma_start(
                            out=mc[:, ts(c, N)], in_=mc_d[i][:, ts(c, N)])
                s["mc"] = mc
                ep = qkpsum.tile([H, N], F32, tag="qk")
                nc.tensor.matmul(ep[:], lhsT=(encw_t[:]), rhs=(xt), start=True, stop=True)
                hT = hpool.tile([H, N], BF16, tag="h")
                nc.scalar.activation(hT[:], ep[:], AF.Relu, bias=encb_t[:])
                s["h"] = hT

            def stage_A(i, p):
                s = st[i]
                hT = s["h"]
                qp = qkpsum.tile([H, N], F32, tag="qk")
                nc.tensor.matmul(qp[:], lhsT=(wq_t[p][:]), rhs=(hT[:]), start=True, stop=True)
                qt = qpool.tile([H, N], BF16, tag="q")
                nc.vector.tensor_scalar(
                    out=qt[:], in0=qp[:], scalar1=bq_t[p][:], scalar2=0.0,
                    op0=OP.add, op1=OP.max,
                )
                s["q"] = qt
                kp = qkpsum.tile([H, N], F32, tag="qk")
                nc.tensor.matmul(kp[:], lhsT=(wk_t[p][:]), rhs=(hT[:]), start=True, stop=True)
                kt = kpool.tile([H, N], BF16, tag="k")
                nc.vector.tensor_scalar(
                    out=kt[:], in0=kp[:], scalar1=bk_t[p][:], scalar2=0.0,
                    op0=OP.add, op1=OP.max,
                )
                s["k"] = kt
                vp = vpsum.tile([128, NCH * H], F32, tag="v")
                nc.tensor.matmul(
                    vp[:], lhsT=(ones_t[0:1, :]), rhs=(bv4_t[p][:]),
                    start=True, stop=False,
                )
                for c in range(NCH):
                    nc.tensor.matmul(
                        vp[:, ts(c, H)], lhsT=(hT[:, ts(c, 128)]), rhs=(wv_t[p][:]),
                        start=False, stop=(c == NCH - 1),
                    )
                vt = vpool.tile([128, NCH * H], BF16, tag="v")
                nc.vector.tensor_scalar_max(vt[:], vp[:], 0.0)
                s["v"] = vt

            def stage_B(i, p):
                s = st[i]
                pt = ppool.tile([128, NCH * N], BF16, tag="p")
                for half in range(NCH // 2):
                    scp = scpsum.tile([128, 2 * N], F32, tag="sc")
                    for cc in range(2):
                        c = 2 * half + cc
                        nc.tensor.matmul(
                            scp[:, ts(cc, N)], lhsT=(s["k"][:, ts(c, 128)]),
                            rhs=(s["q"][:]), start=True,
                            stop=(c not in _ident_set(i)),
                            skip_group_check=True,
                        )
                    for cc in range(2):
                        c = 2 * half + cc
                        if c in _ident_set(i):
                            # fold the additive mask into the scores PSUM
                            # with an identity matmul; exp masks exactly.
                            nc.tensor.matmul(
                                scp[:, ts(cc, N)], lhsT=(ident_t[:]),
                                rhs=(s["mc"][:, ts(c, N)]),
                                start=False, stop=True, skip_group_check=True,
                            )
                    nc.scalar.activation(pt[:, ts(half, 2 * N)], scp[:], AF.Exp)
                    # 0/1-mask this half's GPS chunks right away (one op per
                    # chunk so rowsum/att@v unblock progressively)
                    for c in range(NCH):
                        if c in _ident_set(i) or c // 2 != half:
                            continue
                        if True:
                            nc.gpsimd.tensor_tensor(
                                out=pt[:, ts(c, N)], in0=pt[:, ts(c, N)],
                                in1=s["mc"][:, ts(c, N)], op=OP.mult,
                            )
                s["p"] = pt

            def stage_C(i, p):
                s = st[i]
                rsp = rspsum.tile([128, N], F32, tag="rs")
                otp = otpsum.tile([H, N], F32, tag="ot")
                iset = _ident_set(i)
                order = list(iset) + [c for c in range(NCH) if c not in iset]
                for j, c in enumerate(order):
                    nc.tensor.matmul(
                        rsp[:], lhsT=(ones_t[:]), rhs=(s["p"][:, ts(c, N)]),
                        start=(j == 0), stop=(j == NCH - 1),
                    )
                    nc.tensor.matmul(
                        otp[:], lhsT=(s["v"][:, ts(c, H)]), rhs=(s["p"][:, ts(c, N)]),
                        start=(j == 0), stop=(j == NCH - 1),
                    )
                s["rs"], s["ot"] = rsp, otp

            def stage_D(i, p):
                s = st[i]
                # 1/rowsum = exp(-ln(rowsum)): Ln/Exp share the loaded ACT
                # table set.
                lnr = rpool.tile([H, N], F32, tag="lnr")
                nc.scalar.activation(lnr[:], s["rs"][:], AF.Ln)
                recip = rpool.tile([H, N], F32, tag="recip")
                nc.scalar.activation(recip[:], lnr[:], AF.Exp, scale=-1.0)
                otn = opool.tile([H, N], BF16, tag="otn")
                nc.vector.tensor_tensor(
                    out=otn[:], in0=s["ot"][:], in1=recip[:], op=OP.mult,
                )
                h2p = otpsum.tile([H, N], F32, tag="ot")
                nc.tensor.matmul(h2p[:], lhsT=(wo_t[p][:]), rhs=(otn[:]), start=True, stop=True)
                # overwrite hT in place (all pass-p readers are done)
                nc.vector.tensor_scalar(
                    out=s["h"][:], in0=h2p[:], scalar1=bo_t[p][:],
                    scalar2=0.0, op0=OP.add, op1=OP.max,
                )

            def stage_Hd(i):
                s = st[i]
                yp = qkpsum.tile([A, N], F32, tag="qk")
                nc.tensor.matmul(yp[:], lhsT=(qw_t[:]), rhs=(s["h"][:]), start=True, stop=True)
                y1 = ypool.tile([A, N], F32, tag="y")
                nc.vector.tensor_scalar_add(y1[:], yp[:], qb_t[:])
                nc.sync.dma_start(out=yt_d[i], in_=y1[:])

            def emit(i, sidx):
                if sidx == 0:
                    stage_E(i)
                elif sidx == 9:
                    stage_Hd(i)
                else:
                    p, sub = divmod(sidx - 1, 4)
                    [stage_A, stage_B, stage_C, stage_D][sub](i, p)

            stage_E(0)
            wq_t, wk_t, wv_t, wo_t = [None] * P, [None] * P, [None] * P, [None] * P
            bq_t, bk_t, bv4_t, bo_t = [None] * P, [None] * P, [None] * P, [None] * P
            # ones/bv4 first: the scheduler hoists the dependency-light
            # v-bias preload matmul to the PE program head, so its inputs
            # must arrive first or the whole PE stream waits on them
            ones_t = wtile([128, 128], BF16, "ones", ones_d[:])
            bv4_t[0] = wtile([1, NCH * H], BF16, "bv40", bv4_d[0])
            for p in range(P):
                wq_t[p] = wtile([H, H], BF16, f"wq{p}", wq_d[p])
                wk_t[p] = wtile([H, H], BF16, f"wk{p}", wk_d[p])
                wv_t[p] = wtile([H, H], BF16, f"wv{p}", wv_d[p])
                if p > 0:
                    bv4_t[p] = wtile([1, NCH * H], BF16, f"bv4{p}", bv4_d[p])
                bq_t[p] = wtile([H, 1], F32, f"bq{p}", bq_d[p])
                bk_t[p] = wtile([H, 1], F32, f"bk{p}", bk_d[p])
                if p == 0:
                    ident_t = wtile([128, 128], BF16, "ident", ident_d[:])
                wo_t[p] = wtile([H, H], BF16, f"wo{p}", wo_d[p])
                bo_t[p] = wtile([H, 1], F32, f"bo{p}", bo_d[p])
            qw_t = wtile([H, A], BF16, "qw", qw_d[:])
            qb_t = wtile([A, 1], F32, "qb", qb_d[:])

            # skew 2 in steady state, compressed to 1 for the last three
            # items so the pipeline drains with denser overlap
            NSTAGE = 10
            start = [2 * i for i in range(IPC)]
            start[1] = 1  # item 1 enters early: denser fill while DMAs warm
            for i in range(IPC - 3, IPC):
                start[i] = start[IPC - 4] + (i - (IPC - 4))
            for t in range(start[-1] + NSTAGE):
                for i in range(IPC):  # older (further-along) item first
                    sidx = t - start[i]
                    if 0 <= sidx < NSTAGE:
                        if i == 0 and sidx == 0:
                            continue  # stage_E(0) pre-emitted above
                        emit(i, sidx)

    _spill_excess_waits(nc)
    return nc


_prog_cache = None


def _get_program():
    global _prog_cache
    if _prog_cache is None:
        _prog_cache = build_program()
    return _prog_cache


def _make_in_maps(x, mask, enc_w, enc_b, wv, bv, wk, bk, wq, bq, wo, bo, qw, qb):
    import ml_dtypes
    bf = lambda a: np.ascontiguousarray(np.asarray(a, dtype=np.float32).astype(ml_dtypes.bfloat16))
    f = lambda a: np.ascontiguousarray(np.asarray(a, dtype=np.float32))
    x, mask = f(x), f(mask)
    shared = {
        "enc_w": bf(enc_w),
        "enc_b": f(enc_b).reshape(H, 1),
        "wq": bf(wq),
        "wk": bf(wk),
        "wv": bf(wv),
        "wo": bf(wo),
        "bq": f(bq).reshape(P, H, 1),
        "bk": f(bk).reshape(P, H, 1),
        "bv4": np.ascontiguousarray(np.tile(bf(bv), (1, NCH)).reshape(P, 1, NCH * H)),
        "bo": f(bo).reshape(P, H, 1),
        "qw": bf(qw),
        "ones": np.ones((128, 128), dtype=ml_dtypes.bfloat16),
        "ident": np.eye(128, dtype=ml_dtypes.bfloat16),
        "qb": f(qb).reshape(A, 1),
    }
    # transposed mask, pre-chunked to the on-chip [128, NCH*N] layout:
    # chunk c occupies cols [c*N, (c+1)*N) with partition i holding row
    # m = c*128 + i. Chunks < NIC ship as additive -30000*(1-mask) (for
    # the ident-matmul), the rest as raw 0/1 (GPSIMD multiply).
    maskT = mask.transpose(0, 2, 1)  # [B, m, n]
    maskc = maskT.reshape(B, NCH, 128, N).transpose(0, 2, 1, 3)  # [B,128,c,n]
    maskc = np.ascontiguousarray(maskc).reshape(B, 128, NCH * N)
    maskc = maskc.copy()
    for i_ in range(IPC):
        bidx = np.arange(i_, B, IPC)  # item i_ on every core
        for c in _ident_set(i_):
            maskc[bidx, :, c * N:(c + 1) * N] = (
                -30000.0 * (1.0 - maskc[bidx, :, c * N:(c + 1) * N]))
    maskc = maskc.astype(ml_dtypes.bfloat16)
    # x for all items in one [DIN, IPC*N] block per core
    xb = x.transpose(0, 2, 1).astype(ml_dtypes.bfloat16)  # [B, DIN, N]
    in_maps = []
    for c in range(N_CORES):
        sl = slice(c * IPC, (c + 1) * IPC)
        in_maps.append({
            "xt": np.ascontiguousarray(
                xb[sl].transpose(1, 0, 2).reshape(DIN, IPC * N)),
            "maskc": np.ascontiguousarray(maskc[sl]),
            **shared,
        })
    return in_maps


def run(trace=False, **inputs):
    nc = _get_program()
    in_maps = _make_in_maps(**inputs)
    res = run_bass_kernel_spmd(nc, in_maps, list(range(N_CORES)), trace=trace)
    y = np.concatenate(
        [r["yt"].transpose(0, 2, 1) for r in res.results], axis=0
    ).astype(np.float32)
    return y, res


def kernel(**inputs):
    y, _ = run(trace=False, **inputs)
    return y
